# revision 9
# baseline (speedup 1.0000x reference)
"""Trainium2 Bass kernel for nn_GatedBlock (moe_routing).

Math (reference collapses): the (NB,BS,BS) reshape of weight maps block k to
rows [128k, 128k+128) of weight, so
    out[b, i] = g[b, i // 128] * (x @ W.T)[b, i] + bias[i]
with g = sigmoid(x @ gate_w + gate_b), bottom-8 of 16 gates zeroed per row.

Sharding: output-dim (i) split 8 ways -> 256 rows of W (= 2 gate blocks) per
core.  Per-core inputs (all k-tile-major, partition-contiguous rows):
  small (128, KT, 96) bf16  [x_hi | x_lo | gw_hi | gw_lo]  (3KB rows)
  rhs   (128, KT, 256) bf16 W_shard.T                      (512B/k-tile rows)
  epi   (32, 272) f32       [bias_shard bcast | gate_b[perm] bcast]

Everything runs in bf16 on the tensor engine:
  gate:  16 mms, lhsT = [x_hi|x_lo] (M=64), rhs = [gw_hi|gw_lo] (N=32);
         logits = ps[0:32,0:16] + ps[0:32,16:32] + ps[32:64,0:16] recovers
         x@gw to ~1e-5 (bf16 products are exact, fp32 PSUM accumulate, only
         the x_lo*gw_lo term is dropped) — top-8 ranking is safe (min gate
         margin 3.4e-4).
  main:  16 mms, lhsT = x_hi (M=32), rhs = W k-tile (N=256) — one wide mm
         per k-tile amortizes the ~50ns per-instruction overhead.
W in bf16 gives 1.9e-3 rel err vs the 2e-2 gate and halves the dominant DMA.
DMA: one small DMA + one W-segment DMA per HWDGE queue (big partition rows
amortize the ~60ns/packet engine pacing and the ~0.6us per-DMA turnaround).
"""

import sys

for _p in ("/opt/trn_rl_repo", "/root/.axon_site/_ro/trn_rl_repo"):
    if _p not in sys.path:
        sys.path.append(_p)

import os as _os

import numpy as np

B = 32          # batch
D = 2048        # model dim
NB = 16         # gate blocks
BLK = D // NB   # 128 output rows per gate block
N_CORES = 8
NOUT = D // N_CORES       # 256 output cols per core
KT = D // 128             # 16 k-tiles
NSM = 96                  # small cols: 64 x (hi|lo) + 32 gw (hi|lo)

# k split points for the W DMA segments (round-robin over queues, starting
# with the queue that does NOT carry the small array)
SPLITS = [int(v) for v in _os.environ.get("GATED_SPLITS", "4,8,12").split(",") if v]
DMA_ENGS = _os.environ.get("GATED_ENGS", "sync,scalar").split(",")
SMALL_SPLIT = int(_os.environ.get("GATED_SMALL_SPLIT", "8"))  # 0 = single DMA
MAIN_M = int(_os.environ.get("GATED_MAIN_M", "32"))           # 32 or 64

_compiled = {}


def _build():
    import concourse.bacc as bacc
    import concourse.tile as tile
    import concourse.mybir as mybir

    f32 = mybir.dt.float32
    bf16 = mybir.dt.bfloat16

    nc = bacc.Bacc("TRN2", target_bir_lowering=False, debug=False,
                   num_devices=N_CORES)

    small_d = nc.dram_tensor("small", [128, KT, NSM], bf16, kind="ExternalInput")
    rhs_d = nc.dram_tensor("rhs", [128, KT, NOUT], bf16, kind="ExternalInput")
    epi_d = nc.dram_tensor("epi", [B, NOUT + NB], f32, kind="ExternalInput")
    out_d = nc.dram_tensor("out", [B, NOUT], f32, kind="ExternalOutput")

    with tile.TileContext(nc) as tc:
        with (
            tc.tile_pool(name="sb", bufs=1) as sb,
            tc.tile_pool(name="ps", bufs=1, space="PSUM") as psp,
        ):
            small = sb.tile([128, KT, NSM], bf16, name="small_sb", tag="small_sb")
            rhs = sb.tile([128, KT, NOUT], bf16, name="rhs_sb", tag="rhs_sb")
            epi = sb.tile([B, NOUT + NB], f32, name="epi_sb", tag="epi_sb")
            t1 = sb.tile([B, NB], f32, name="t1", tag="t1")
            graw = sb.tile([B, NB], f32, name="graw", tag="graw")
            g = sb.tile([B, NB], f32, name="g", tag="g")
            m8 = sb.tile([B, 8], f32, name="m8", tag="m8")
            rep = sb.tile([B, NB], f32, name="rep", tag="rep")
            gk = sb.tile([B, NB], f32, name="gk", tag="gk")
            outt = sb.tile([B, NOUT], f32, name="outt", tag="outt")
            ps_g = psp.tile([2 * B, 2 * NB], f32, name="ps_g", tag="ps_g")
            ps_m = psp.tile([MAIN_M, NOUT], f32, name="ps_m", tag="ps_m")

            engs = [getattr(nc, e) for e in DMA_ENGS]

            # queue 0: the small array (gate + stationary data) first;
            # queue 1: epi then the first W segment.  W segments round-robin
            # starting on queue 1 so queue 0's small lands first.
            if SMALL_SPLIT:
                engs[0].dma_start(small[:, :SMALL_SPLIT, :],
                                  small_d.ap()[:, :SMALL_SPLIT, :])
                engs[0].dma_start(small[:, SMALL_SPLIT:, :],
                                  small_d.ap()[:, SMALL_SPLIT:, :])
            else:
                engs[0].dma_start(small[:], small_d.ap())
            engs[1 % len(engs)].dma_start(epi[:], epi_d.ap())
            bounds = [0] + SPLITS + [KT]
            for si in range(len(bounds) - 1):
                k0, k1 = bounds[si], bounds[si + 1]
                engs[(si + 1) % len(engs)].dma_start(
                    rhs[:, k0:k1, :], rhs_d.ap()[:, k0:k1, :])

            # gate linear: M=64 ([x_hi|x_lo]) x N=32 ([gw_hi|gw_lo])
            for t in range(KT):
                nc.tensor.matmul(
                    ps_g[:], small[:, t, 0:64], small[:, t, 64:NSM],
                    start=(t == 0), stop=(t == KT - 1),
                )

            # logits = hi*hi + hi*lo + lo*hi + gate_b, then sigmoid/top-8
            # (chained so each tensor_tensor reads at most one PSUM input)
            nc.vector.tensor_add(t1[:], ps_g[0:B, 0:NB], epi[:, NOUT:NOUT + NB])
            nc.vector.tensor_add(t1[:], ps_g[0:B, NB:2 * NB], t1[:])
            nc.vector.tensor_add(graw[:], ps_g[B:2 * B, 0:NB], t1[:])
            nc.scalar.activation(g[:], graw[:],
                                 mybir.ActivationFunctionType.Sigmoid)
            nc.vector.max(m8[:], g[:])
            nc.vector.match_replace(rep[:], m8[:], g[:], 0.0)
            nc.vector.tensor_sub(gk[:], g[:], rep[:])

            # main matmul: one N=256 mm per k-tile, x_hi (or [x_hi|x_lo])
            # stationary; with MAIN_M=64 rows 32-63 are x_lo*W (unused)
            for t in range(KT):
                nc.tensor.matmul(
                    ps_m[:], small[:, t, 0:MAIN_M], rhs[:, t, :],
                    start=(t == 0), stop=(t == KT - 1),
                )

            # out = psum * g[block] + bias, in halves so the first store
            # overlaps the second STT
            nh = NOUT // BLK
            for h in range(nh):
                sl = slice(h * BLK, (h + 1) * BLK)
                nc.vector.scalar_tensor_tensor(
                    outt[:, sl], ps_m[0:B, sl], gk[:, h:h + 1], epi[:, sl],
                    mybir.AluOpType.mult, mybir.AluOpType.add,
                )
                engs[(h + 1) % len(engs)].dma_start(out_d.ap()[:, sl], outt[:, sl])

    nc.compile()
    return nc


def _tile_major(a):
    """(D, n) -> (128, KT, n) k-tile-major contiguous."""
    n = a.shape[1]
    return np.ascontiguousarray(a.reshape(KT, 128, n).transpose(1, 0, 2))


def _hi_lo(a):
    import ml_dtypes
    hi = a.astype(ml_dtypes.bfloat16)
    lo = (a - hi.astype(np.float32)).astype(ml_dtypes.bfloat16)
    return hi, lo


def build_in_maps(x, gate_w, gate_b, weight, bias):
    import ml_dtypes

    x = np.asarray(x, dtype=np.float32)
    gate_w = np.asarray(gate_w, dtype=np.float32)
    gate_b = np.asarray(gate_b, dtype=np.float32)
    weight = np.asarray(weight, dtype=np.float32)
    bias = np.asarray(bias, dtype=np.float32)

    x_hi, x_lo = _hi_lo(np.ascontiguousarray(x.T))               # (2048, 32)
    in_maps = []
    for c in range(N_CORES):
        perm = [2 * c, 2 * c + 1] + [k for k in range(NB)
                                     if k not in (2 * c, 2 * c + 1)]
        gw_hi, gw_lo = _hi_lo(gate_w[:, perm])                   # (2048, 16)
        small = np.concatenate([x_hi, x_lo, gw_hi, gw_lo], axis=1)  # (2048, 96)
        w_shard = np.ascontiguousarray(weight[c * NOUT:(c + 1) * NOUT, :].T)
        epi = np.concatenate([
            np.broadcast_to(bias[c * NOUT:(c + 1) * NOUT], (B, NOUT)),
            np.broadcast_to(gate_b[perm], (B, NB)),
        ], axis=1).astype(np.float32)
        in_maps.append({
            "small": _tile_major(small),
            "rhs": _tile_major(w_shard.astype(ml_dtypes.bfloat16)),
            "epi": np.ascontiguousarray(epi),
        })
    return in_maps


def _ensure_ntff_hook():
    """If a caller sets BASS_TRACE, run_bass_kernel_spmd imports
    antenv.axon_hooks, which is missing in this image; provide a working
    ctypes-backed stub so tracing degrades gracefully instead of raising."""
    try:
        from antenv.axon_hooks import get_axon_ntff_profile_hook  # noqa: F401
        return
    except ImportError:
        pass
    import contextlib
    import ctypes
    import types

    try:
        lib = ctypes.CDLL("/opt/axon/libaxon_pjrt.so")
        assert hasattr(lib, "axon_start_nrt_profile")
        lib.axon_start_nrt_profile.argtypes = [
            ctypes.POINTER(ctypes.c_int64), ctypes.c_size_t]
        lib.axon_start_nrt_profile.restype = ctypes.c_int64
        lib.axon_stop_nrt_profile.argtypes = [ctypes.c_char_p]
        lib.axon_stop_nrt_profile.restype = ctypes.c_int64

        @contextlib.contextmanager
        def _hook(output_dir, device_ids):
            import jax
            jax.devices()
            if device_ids:
                ids = (ctypes.c_int64 * len(device_ids))(*device_ids)
                rc = lib.axon_start_nrt_profile(ids, len(device_ids))
            else:
                rc = lib.axon_start_nrt_profile(None, 0)
            if rc != 0:
                raise RuntimeError(f"axon_start_nrt_profile rc={rc}")
            try:
                yield
            finally:
                lib.axon_stop_nrt_profile(str(output_dir).encode())

        hook = _hook
    except Exception:
        hook = None

    mod = types.ModuleType("antenv.axon_hooks")
    mod.get_axon_ntff_profile_hook = lambda: hook
    mod.set_axon_ntff_profile_hook = lambda h: None
    sys.modules["antenv.axon_hooks"] = mod


MODE = "v3"  # single variant; kept for test.py compatibility


def kernel(x, gate_w, gate_b, weight, bias):
    _ensure_ntff_hook()
    from concourse.bass_utils import run_bass_kernel_spmd

    if MODE not in _compiled:
        _compiled[MODE] = _build()
    nc = _compiled[MODE]

    in_maps = build_in_maps(x, gate_w, gate_b, weight, bias)
    res = run_bass_kernel_spmd(nc, in_maps, list(range(N_CORES)))
    out = np.concatenate([res.results[c]["out"] for c in range(N_CORES)], axis=1)
    return out.astype(np.float32)


# revision 11
# speedup vs baseline: 1.0769x; 1.0769x over previous
"""Trainium2 Bass kernel for nn_GatedBlock (moe_routing).

Math (reference collapses): the (NB,BS,BS) reshape of weight maps block k to
rows [128k, 128k+128) of weight, so
    out[b, i] = g[b, i // 128] * (x @ W.T)[b, i] + bias[i]
with g = sigmoid(x @ gate_w + gate_b), bottom-8 of 16 gates zeroed per row.

Sharding: output-dim (i) split 8 ways -> 256 rows of W (= 2 gate blocks) per
core.  Per-core inputs (all k-tile-major, partition-contiguous rows):
  small (128, KT, 96) bf16  [x_hi | x_lo | gw_hi | gw_lo]  (3KB rows)
  rhs   (128, KT, 256) bf16 W_shard.T                      (512B/k-tile rows)
  epi   (32, 272) f32       [bias_shard bcast | gate_b[perm] bcast]

Everything runs in bf16 on the tensor engine:
  gate:  16 mms, lhsT = [x_hi|x_lo] (M=64), rhs = [gw_hi|gw_lo] (N=32);
         logits = ps[0:32,0:16] + ps[0:32,16:32] + ps[32:64,0:16] recovers
         x@gw to ~1e-5 (bf16 products are exact, fp32 PSUM accumulate, only
         the x_lo*gw_lo term is dropped) — top-8 ranking is safe (min gate
         margin 3.4e-4).
  main:  16 mms, lhsT = x_hi (M=32), rhs = W k-tile (N=256) — one wide mm
         per k-tile amortizes the ~50ns per-instruction overhead.
W in bf16 gives 1.9e-3 rel err vs the 2e-2 gate and halves the dominant DMA.
DMA: one small DMA + one W-segment DMA per HWDGE queue (big partition rows
amortize the ~60ns/packet engine pacing and the ~0.6us per-DMA turnaround).
"""

import sys

for _p in ("/opt/trn_rl_repo", "/root/.axon_site/_ro/trn_rl_repo"):
    if _p not in sys.path:
        sys.path.append(_p)

import os as _os

import numpy as np

B = 32          # batch
D = 2048        # model dim
NB = 16         # gate blocks
BLK = D // NB   # 128 output rows per gate block
N_CORES = 8
NOUT = D // N_CORES       # 256 output cols per core
KT = D // 128             # 16 k-tiles
NSM = 96                  # small cols: 64 x (hi|lo) + 32 gw (hi|lo)

# k split points for the W DMA segments and per-segment queue index
SPLITS = [int(v) for v in _os.environ.get("GATED_SPLITS", "6,12").split(",") if v]
SEGQ = [int(v) for v in _os.environ.get("GATED_SEGQ", "0,1,1").split(",") if v != ""]
DMA_ENGS = _os.environ.get("GATED_ENGS", "sync,scalar").split(",")
SMALL_SPLIT = int(_os.environ.get("GATED_SMALL_SPLIT", "0"))  # 0 = single DMA
MAIN_M = int(_os.environ.get("GATED_MAIN_M", "32"))           # 32 or 64
EPI_Q = int(_os.environ.get("GATED_EPI_Q", "1"))
EPI_LAST = _os.environ.get("GATED_EPI_LAST", "1") == "1"

_compiled = {}


def _build():
    import concourse.bacc as bacc
    import concourse.tile as tile
    import concourse.mybir as mybir

    f32 = mybir.dt.float32
    bf16 = mybir.dt.bfloat16

    nc = bacc.Bacc("TRN2", target_bir_lowering=False, debug=False,
                   num_devices=N_CORES)

    small_d = nc.dram_tensor("small", [128, KT, NSM], bf16, kind="ExternalInput")
    rhs_d = nc.dram_tensor("rhs", [128, KT, NOUT], bf16, kind="ExternalInput")
    epi_d = nc.dram_tensor("epi", [B, NOUT + NB], f32, kind="ExternalInput")
    out_d = nc.dram_tensor("out", [B, NOUT], f32, kind="ExternalOutput")

    with tile.TileContext(nc) as tc:
        with (
            tc.tile_pool(name="sb", bufs=1) as sb,
            tc.tile_pool(name="ps", bufs=1, space="PSUM") as psp,
        ):
            small = sb.tile([128, KT, NSM], bf16, name="small_sb", tag="small_sb")
            rhs = sb.tile([128, KT, NOUT], bf16, name="rhs_sb", tag="rhs_sb")
            epi = sb.tile([B, NOUT + NB], f32, name="epi_sb", tag="epi_sb")
            t1 = sb.tile([B, NB], f32, name="t1", tag="t1")
            graw = sb.tile([B, NB], f32, name="graw", tag="graw")
            g = sb.tile([B, NB], f32, name="g", tag="g")
            m8 = sb.tile([B, 8], f32, name="m8", tag="m8")
            rep = sb.tile([B, NB], f32, name="rep", tag="rep")
            gk = sb.tile([B, NB], f32, name="gk", tag="gk")
            outt = sb.tile([B, NOUT], f32, name="outt", tag="outt")
            ps_g = psp.tile([2 * B, 2 * NB], f32, name="ps_g", tag="ps_g")
            ps_m = psp.tile([MAIN_M, NOUT], f32, name="ps_m", tag="ps_m")

            engs = [getattr(nc, e) for e in DMA_ENGS]

            # queue 0: the small array (gate + stationary data) first, then
            # its share of W segments; queue 1: W segments, epi last (epi is
            # only needed by the gate-bias add and the epilogue).
            if SMALL_SPLIT:
                engs[0].dma_start(small[:, :SMALL_SPLIT, :],
                                  small_d.ap()[:, :SMALL_SPLIT, :])
                engs[0].dma_start(small[:, SMALL_SPLIT:, :],
                                  small_d.ap()[:, SMALL_SPLIT:, :])
            else:
                engs[0].dma_start(small[:], small_d.ap())
            if not EPI_LAST:
                engs[EPI_Q % len(engs)].dma_start(epi[:], epi_d.ap())
            bounds = [0] + SPLITS + [KT]
            for si in range(len(bounds) - 1):
                k0, k1 = bounds[si], bounds[si + 1]
                engs[SEGQ[si] % len(engs)].dma_start(
                    rhs[:, k0:k1, :], rhs_d.ap()[:, k0:k1, :])
            if EPI_LAST:
                engs[EPI_Q % len(engs)].dma_start(epi[:], epi_d.ap())

            # gate linear: M=64 ([x_hi|x_lo]) x N=32 ([gw_hi|gw_lo])
            for t in range(KT):
                nc.tensor.matmul(
                    ps_g[:], small[:, t, 0:64], small[:, t, 64:NSM],
                    start=(t == 0), stop=(t == KT - 1),
                )

            # logits = hi*hi + hi*lo + lo*hi + gate_b, then sigmoid/top-8
            # (chained so each tensor_tensor reads at most one PSUM input)
            nc.vector.tensor_add(t1[:], ps_g[0:B, 0:NB], epi[:, NOUT:NOUT + NB])
            nc.vector.tensor_add(t1[:], ps_g[0:B, NB:2 * NB], t1[:])
            nc.vector.tensor_add(graw[:], ps_g[B:2 * B, 0:NB], t1[:])
            nc.scalar.activation(g[:], graw[:],
                                 mybir.ActivationFunctionType.Sigmoid)
            nc.vector.max(m8[:], g[:])
            nc.vector.match_replace(rep[:], m8[:], g[:], 0.0)
            nc.vector.tensor_sub(gk[:], g[:], rep[:])

            # main matmul: one N=256 mm per k-tile, x_hi (or [x_hi|x_lo])
            # stationary; with MAIN_M=64 rows 32-63 are x_lo*W (unused)
            for t in range(KT):
                nc.tensor.matmul(
                    ps_m[:], small[:, t, 0:MAIN_M], rhs[:, t, :],
                    start=(t == 0), stop=(t == KT - 1),
                )

            # out = psum * g[block] + bias, in halves so the first store
            # overlaps the second STT
            nh = NOUT // BLK
            for h in range(nh):
                sl = slice(h * BLK, (h + 1) * BLK)
                nc.vector.scalar_tensor_tensor(
                    outt[:, sl], ps_m[0:B, sl], gk[:, h:h + 1], epi[:, sl],
                    mybir.AluOpType.mult, mybir.AluOpType.add,
                )
                engs[(h + 1) % len(engs)].dma_start(out_d.ap()[:, sl], outt[:, sl])

    nc.compile()
    return nc


def _tile_major(a):
    """(D, n) -> (128, KT, n) k-tile-major contiguous."""
    n = a.shape[1]
    return np.ascontiguousarray(a.reshape(KT, 128, n).transpose(1, 0, 2))


def _hi_lo(a):
    import ml_dtypes
    hi = a.astype(ml_dtypes.bfloat16)
    lo = (a - hi.astype(np.float32)).astype(ml_dtypes.bfloat16)
    return hi, lo


def build_in_maps(x, gate_w, gate_b, weight, bias):
    import ml_dtypes

    x = np.asarray(x, dtype=np.float32)
    gate_w = np.asarray(gate_w, dtype=np.float32)
    gate_b = np.asarray(gate_b, dtype=np.float32)
    weight = np.asarray(weight, dtype=np.float32)
    bias = np.asarray(bias, dtype=np.float32)

    x_hi, x_lo = _hi_lo(np.ascontiguousarray(x.T))               # (2048, 32)
    in_maps = []
    for c in range(N_CORES):
        perm = [2 * c, 2 * c + 1] + [k for k in range(NB)
                                     if k not in (2 * c, 2 * c + 1)]
        gw_hi, gw_lo = _hi_lo(gate_w[:, perm])                   # (2048, 16)
        small = np.concatenate([x_hi, x_lo, gw_hi, gw_lo], axis=1)  # (2048, 96)
        w_shard = np.ascontiguousarray(weight[c * NOUT:(c + 1) * NOUT, :].T)
        epi = np.concatenate([
            np.broadcast_to(bias[c * NOUT:(c + 1) * NOUT], (B, NOUT)),
            np.broadcast_to(gate_b[perm], (B, NB)),
        ], axis=1).astype(np.float32)
        in_maps.append({
            "small": _tile_major(small),
            "rhs": _tile_major(w_shard.astype(ml_dtypes.bfloat16)),
            "epi": np.ascontiguousarray(epi),
        })
    return in_maps


def _ensure_ntff_hook():
    """If a caller sets BASS_TRACE, run_bass_kernel_spmd imports
    antenv.axon_hooks, which is missing in this image; provide a working
    ctypes-backed stub so tracing degrades gracefully instead of raising."""
    try:
        from antenv.axon_hooks import get_axon_ntff_profile_hook  # noqa: F401
        return
    except ImportError:
        pass
    import contextlib
    import ctypes
    import types

    try:
        lib = ctypes.CDLL("/opt/axon/libaxon_pjrt.so")
        assert hasattr(lib, "axon_start_nrt_profile")
        lib.axon_start_nrt_profile.argtypes = [
            ctypes.POINTER(ctypes.c_int64), ctypes.c_size_t]
        lib.axon_start_nrt_profile.restype = ctypes.c_int64
        lib.axon_stop_nrt_profile.argtypes = [ctypes.c_char_p]
        lib.axon_stop_nrt_profile.restype = ctypes.c_int64

        @contextlib.contextmanager
        def _hook(output_dir, device_ids):
            import jax
            jax.devices()
            if device_ids:
                ids = (ctypes.c_int64 * len(device_ids))(*device_ids)
                rc = lib.axon_start_nrt_profile(ids, len(device_ids))
            else:
                rc = lib.axon_start_nrt_profile(None, 0)
            if rc != 0:
                raise RuntimeError(f"axon_start_nrt_profile rc={rc}")
            try:
                yield
            finally:
                lib.axon_stop_nrt_profile(str(output_dir).encode())

        hook = _hook
    except Exception:
        hook = None

    mod = types.ModuleType("antenv.axon_hooks")
    mod.get_axon_ntff_profile_hook = lambda: hook
    mod.set_axon_ntff_profile_hook = lambda h: None
    sys.modules["antenv.axon_hooks"] = mod


MODE = "v3"  # single variant; kept for test.py compatibility


def kernel(x, gate_w, gate_b, weight, bias):
    _ensure_ntff_hook()
    from concourse.bass_utils import run_bass_kernel_spmd

    if MODE not in _compiled:
        _compiled[MODE] = _build()
    nc = _compiled[MODE]

    in_maps = build_in_maps(x, gate_w, gate_b, weight, bias)
    res = run_bass_kernel_spmd(nc, in_maps, list(range(N_CORES)))
    out = np.concatenate([res.results[c]["out"] for c in range(N_CORES)], axis=1)
    return out.astype(np.float32)


# revision 26
# speedup vs baseline: 1.0890x; 1.0112x over previous
"""Trainium2 Bass kernel for nn_GatedBlock (moe_routing).

Math (reference collapses): the (NB,BS,BS) reshape of weight maps block k to
rows [128k, 128k+128) of weight, so
    out[b, i] = g[b, i // 128] * (x @ W.T)[b, i] + bias[i]
with g = sigmoid(x @ gate_w + gate_b), bottom-8 of 16 gates zeroed per row.

Sharding: output-dim (i) split 8 ways -> 256 rows of W (= 2 gate blocks) per
core.  Per-core inputs (all k-tile-major, partition-contiguous rows):
  small (128, KT, 96) bf16  [x_hi | x_lo | gw_hi | gw_lo]  (3KB rows)
  rhs   (128, KT, 256) bf16 W_shard.T                      (512B/k-tile rows)
  epi   (32, 272) f32       [bias_shard bcast | gate_b[perm] bcast]

Everything runs in bf16 on the tensor engine:
  gate:  16 mms, lhsT = [x_hi|x_lo] (M=64), rhs = [gw_hi|gw_lo] (N=32);
         logits = ps[0:32,0:16] + ps[0:32,16:32] + ps[32:64,0:16] recovers
         x@gw to ~1e-5 (bf16 products are exact, fp32 PSUM accumulate, only
         the x_lo*gw_lo term is dropped) — top-8 ranking is safe (min gate
         margin 3.4e-4).
  main:  16 mms, lhsT = x_hi (M=32), rhs = W k-tile (N=256).
W in bf16 gives 1.9e-3 rel err vs the 2e-2 gate and halves the dominant DMA.
The DMA plan (which queue carries which k-range, in what order) is data for
the build so it can be swept; W arrival order should match the matmuls'
k-order consumption.
"""

import sys

for _p in ("/opt/trn_rl_repo", "/root/.axon_site/_ro/trn_rl_repo"):
    if _p not in sys.path:
        sys.path.append(_p)

import os as _os

import numpy as np

B = 32          # batch
D = 2048        # model dim
NB = 16         # gate blocks
BLK = D // NB   # 128 output rows per gate block
N_CORES = 8
NOUT = D // N_CORES       # 256 output cols per core
KT = D // 128             # 16 k-tiles
NSM = 96                  # small cols: 64 x (hi|lo) + 32 gw (hi|lo)

# DMA plan: per queue (sync=0, scalar=1), ordered entries
#   ("small", k0, k1) | ("rhs", k0, k1) | ("epi",)
# out DMAs always go one half per queue at the end.
DEFAULT_PLAN = {
    0: [("small", 0, KT), ("rhs", 8, 12), ("rhs", 12, 16), ("epib",)],
    1: [("rhs", 0, 4), ("rhs", 4, 8), ("epi",)],
}
DEFAULT_MAIN_M = 32
DEFAULT_ORIENT = "w"   # "x": x stationary (psum [32,256]); "w": W stationary

_compiled = {}


def _build(plan, main_m, probe=False, orient=DEFAULT_ORIENT):
    import concourse.bacc as bacc
    import concourse.tile as tile
    import concourse.mybir as mybir

    f32 = mybir.dt.float32
    bf16 = mybir.dt.bfloat16

    nc = bacc.Bacc("TRN2", target_bir_lowering=False, debug=False,
                   num_devices=N_CORES)

    small_d = nc.dram_tensor("small", [128, KT, NSM], bf16, kind="ExternalInput")
    rhs_d = nc.dram_tensor("rhs", [128, KT, NOUT], bf16, kind="ExternalInput")
    epi_d = nc.dram_tensor("epi", [B, NOUT + NB], f32, kind="ExternalInput")
    if orient == "w":
        epib_d = nc.dram_tensor("epib", [BLK, 2], f32, kind="ExternalInput")
        out_d = nc.dram_tensor("out", [BLK, 2 * B], f32, kind="ExternalOutput")
    else:
        out_d = nc.dram_tensor("out", [B, NOUT], f32, kind="ExternalOutput")

    with tile.TileContext(nc) as tc:
        with (
            tc.tile_pool(name="sb", bufs=1) as sb,
            tc.tile_pool(name="ps", bufs=1, space="PSUM") as psp,
        ):
            small = sb.tile([128, KT, NSM], bf16, name="small_sb", tag="small_sb")
            rhs = sb.tile([128, KT, NOUT], bf16, name="rhs_sb", tag="rhs_sb")
            epi = sb.tile([B, NOUT + NB], f32, name="epi_sb", tag="epi_sb")
            t1 = sb.tile([B, NB], f32, name="t1", tag="t1")
            graw = sb.tile([B, NB], f32, name="graw", tag="graw")
            g = sb.tile([B, NB], f32, name="g", tag="g")
            m8 = sb.tile([B, 8], f32, name="m8", tag="m8")
            rep = sb.tile([B, NB], f32, name="rep", tag="rep")
            gk = sb.tile([B, NB], f32, name="gk", tag="gk")
            ps_g = psp.tile([2 * B, 2 * NB], f32, name="ps_g", tag="ps_g")
            if orient == "w":
                ones = sb.tile([1, BLK], f32, name="ones", tag="ones")
                gkp = sb.tile([B, B], f32, name="gkp", tag="gkp")
                gkT = sb.tile([B, B], f32, name="gkT", tag="gkT")
                gkp1 = sb.tile([B, B], f32, name="gkp1", tag="gkp1")
                gkT1 = sb.tile([B, B], f32, name="gkT1", tag="gkT1")
                gbc = [sb.tile([BLK, B], f32, name=f"gbc{h}", tag=f"gbc{h}")
                       for h in range(2)]
                outw = sb.tile([BLK, 2 * B], f32, name="outw", tag="outw")
                epib = sb.tile([BLK, 2], f32, name="epib_sb", tag="epib_sb")
                ps_w = [psp.tile([BLK, B], f32, name=f"ps_w{h}", tag=f"ps_w{h}")
                        for h in range(2)]
                ps_b = [psp.tile([BLK, B], f32, name=f"ps_b{h}", tag=f"ps_b{h}")
                        for h in range(2)]
                nc.gpsimd.memset(ones[:], 1.0)
            else:
                outt = sb.tile([B, NOUT], f32, name="outt", tag="outt")
                ps_m = psp.tile([main_m, NOUT], f32, name="ps_m", tag="ps_m")

            engs = [nc.sync, nc.scalar]

            for q, entries in sorted(plan.items()):
                for e in entries:
                    if e[0] == "small":
                        _, k0, k1 = e
                        engs[q].dma_start(small[:, k0:k1, :],
                                          small_d.ap()[:, k0:k1, :])
                    elif e[0] == "rhs":
                        _, k0, k1 = e
                        engs[q].dma_start(rhs[:, k0:k1, :],
                                          rhs_d.ap()[:, k0:k1, :])
                    elif e[0] == "epi":
                        engs[q].dma_start(epi[:], epi_d.ap())
                    elif e[0] == "epib":
                        if orient == "w":
                            engs[q].dma_start(epib[:], epib_d.ap())
                    else:
                        raise ValueError(e)

            # gate linear: M=64 ([x_hi|x_lo]) x N=32 ([gw_hi|gw_lo])
            for t in range(KT):
                nc.tensor.matmul(
                    ps_g[:], small[:, t, 0:64], small[:, t, 64:NSM],
                    start=(t == 0), stop=(t == KT - 1),
                )

            # logits = hi*hi + hi*lo + lo*hi + gate_b, then sigmoid/top-8
            # (chained so each tensor_tensor reads at most one PSUM input)
            nc.vector.tensor_add(t1[:], ps_g[0:B, 0:NB], epi[:, NOUT:NOUT + NB])
            nc.vector.tensor_add(t1[:], ps_g[0:B, NB:2 * NB], t1[:])
            nc.vector.tensor_add(graw[:], ps_g[B:2 * B, 0:NB], t1[:])
            nc.scalar.activation(g[:], graw[:],
                                 mybir.ActivationFunctionType.Sigmoid)
            nc.vector.max(m8[:], g[:])
            nc.vector.match_replace(rep[:], m8[:], g[:], 0.0)
            nc.vector.tensor_sub(gk[:], g[:], rep[:])

            if orient == "w":
                # transpose gk to rows (32x32 DVE block transposes) so each
                # block's gate row lands at partition 0 and can be
                # partition-broadcast via a K=1 ones-matmul; all of this runs
                # while the main mms stream.  (DVE operands must start at
                # partition 0, hence a second transpose with block 1's gate
                # column shifted into column 0.)
                nc.vector.tensor_copy(gkp[:, 0:NB], gk[:])
                nc.vector.tensor_copy(gkp[:, NB:B], gk[:])
                nc.vector.transpose(gkT[:], gkp[:])
                nc.vector.tensor_copy(gkp1[:, 0:NB], gk[:])
                nc.vector.tensor_copy(gkp1[:, NB:B], gk[:])
                nc.vector.tensor_copy(gkp1[:, 0:1], gk[:, 1:2])
                nc.vector.transpose(gkT1[:], gkp1[:])

                # main matmul: W k-tile stationary (M=128, ~4 cols/cycle
                # LDWEIGHTS ingest), x_hi moving (N=32) -> psum is out.T
                for t in range(KT):
                    for h in range(2):
                        nc.tensor.matmul(
                            ps_w[h][:], rhs[:, t, h * BLK:(h + 1) * BLK],
                            small[:, t, 0:B],
                            start=(t == 0), stop=(t == KT - 1),
                        )
                # partition-broadcast each block's gate row
                nc.tensor.matmul(ps_b[0][:], ones[0:1, :], gkT[0:1, 0:B],
                                 start=True, stop=True)
                nc.tensor.matmul(ps_b[1][:], ones[0:1, :], gkT1[0:1, 0:B],
                                 start=True, stop=True)
            else:
                # main matmul: one N=256 mm per k-tile, x_hi stationary
                for t in range(KT):
                    nc.tensor.matmul(
                        ps_m[:], small[:, t, 0:main_m], rhs[:, t, :],
                        start=(t == 0), stop=(t == KT - 1),
                    )

            if probe:
                # timing probe: W-stationary (M=128) matmul segment, unused
                # result kept alive by a dummy copy
                ps_p = psp.tile([128, B], f32, name="ps_p", tag="ps_p")
                dmy = sb.tile([128, B], f32, name="dmy", tag="dmy")
                n = 0
                for t in range(KT):
                    for blk in range(2):
                        nc.tensor.matmul(
                            ps_p[:], rhs[:, t, blk * BLK:(blk + 1) * BLK],
                            small[:, t, 0:B],
                            start=(n == 0), stop=(n == 2 * KT - 1),
                        )
                        n += 1
                nc.vector.tensor_copy(dmy[:], ps_p[:])

            if orient == "w":
                # out.T = ps_w * gbc + bias (bias is per-partition here);
                # each block's store issues as soon as that block is ready
                for h in range(2):
                    sl = slice(h * B, (h + 1) * B)
                    nc.vector.tensor_copy(gbc[h][:], ps_b[h][:])
                    nc.vector.tensor_mul(outw[:, sl], ps_w[h][:], gbc[h][:])
                    nc.vector.tensor_scalar_add(outw[:, sl], outw[:, sl],
                                                epib[:, h:h + 1])
                    engs[(h + 1) % 2].dma_start(out_d.ap()[:, sl], outw[:, sl])
            else:
                # out = psum * g[block] + bias, in halves so the first store
                # overlaps the second STT
                nh = NOUT // BLK
                for h in range(nh):
                    sl = slice(h * BLK, (h + 1) * BLK)
                    nc.vector.scalar_tensor_tensor(
                        outt[:, sl], ps_m[0:B, sl], gk[:, h:h + 1], epi[:, sl],
                        mybir.AluOpType.mult, mybir.AluOpType.add,
                    )
                    engs[(h + 1) % 2].dma_start(out_d.ap()[:, sl], outt[:, sl])

    nc.compile()
    return nc


def get_nc(plan=None, main_m=None, probe=False):
    plan = plan if plan is not None else DEFAULT_PLAN
    main_m = main_m if main_m is not None else DEFAULT_MAIN_M
    key = (repr(sorted(plan.items())), main_m, probe)
    if key not in _compiled:
        _compiled[key] = _build(plan, main_m, probe)
    return _compiled[key]


def _tile_major(a):
    """(D, n) -> (128, KT, n) k-tile-major contiguous."""
    n = a.shape[1]
    return np.ascontiguousarray(a.reshape(KT, 128, n).transpose(1, 0, 2))


def _hi_lo(a):
    import ml_dtypes
    hi = a.astype(ml_dtypes.bfloat16)
    lo = (a - hi.astype(np.float32)).astype(ml_dtypes.bfloat16)
    return hi, lo


def build_in_maps(x, gate_w, gate_b, weight, bias):
    import ml_dtypes

    x = np.asarray(x, dtype=np.float32)
    gate_w = np.asarray(gate_w, dtype=np.float32)
    gate_b = np.asarray(gate_b, dtype=np.float32)
    weight = np.asarray(weight, dtype=np.float32)
    bias = np.asarray(bias, dtype=np.float32)

    x_hi, x_lo = _hi_lo(np.ascontiguousarray(x.T))               # (2048, 32)
    in_maps = []
    for c in range(N_CORES):
        perm = [2 * c, 2 * c + 1] + [k for k in range(NB)
                                     if k not in (2 * c, 2 * c + 1)]
        gw_hi, gw_lo = _hi_lo(gate_w[:, perm])                   # (2048, 16)
        small = np.concatenate([x_hi, x_lo, gw_hi, gw_lo], axis=1)  # (2048, 96)
        w_shard = np.ascontiguousarray(weight[c * NOUT:(c + 1) * NOUT, :].T)
        epi = np.concatenate([
            np.broadcast_to(bias[c * NOUT:(c + 1) * NOUT], (B, NOUT)),
            np.broadcast_to(gate_b[perm], (B, NB)),
        ], axis=1).astype(np.float32)
        m = {
            "small": _tile_major(small),
            "rhs": _tile_major(w_shard.astype(ml_dtypes.bfloat16)),
            "epi": np.ascontiguousarray(epi),
        }
        if DEFAULT_ORIENT == "w":
            bs = bias[c * NOUT:(c + 1) * NOUT]
            m["epib"] = np.ascontiguousarray(
                np.stack([bs[0:BLK], bs[BLK:NOUT]], axis=1).astype(np.float32))
        in_maps.append(m)
    return in_maps


def _ensure_ntff_hook():
    """If a caller sets BASS_TRACE, run_bass_kernel_spmd imports
    antenv.axon_hooks, which is missing in this image; provide a working
    ctypes-backed stub so tracing degrades gracefully instead of raising."""
    try:
        from antenv.axon_hooks import get_axon_ntff_profile_hook  # noqa: F401
        return
    except ImportError:
        pass
    import contextlib
    import ctypes
    import types

    try:
        lib = ctypes.CDLL("/opt/axon/libaxon_pjrt.so")
        assert hasattr(lib, "axon_start_nrt_profile")
        lib.axon_start_nrt_profile.argtypes = [
            ctypes.POINTER(ctypes.c_int64), ctypes.c_size_t]
        lib.axon_start_nrt_profile.restype = ctypes.c_int64
        lib.axon_stop_nrt_profile.argtypes = [ctypes.c_char_p]
        lib.axon_stop_nrt_profile.restype = ctypes.c_int64

        @contextlib.contextmanager
        def _hook(output_dir, device_ids):
            import jax
            jax.devices()
            if device_ids:
                ids = (ctypes.c_int64 * len(device_ids))(*device_ids)
                rc = lib.axon_start_nrt_profile(ids, len(device_ids))
            else:
                rc = lib.axon_start_nrt_profile(None, 0)
            if rc != 0:
                raise RuntimeError(f"axon_start_nrt_profile rc={rc}")
            try:
                yield
            finally:
                lib.axon_stop_nrt_profile(str(output_dir).encode())

        hook = _hook
    except Exception:
        hook = None

    mod = types.ModuleType("antenv.axon_hooks")
    mod.get_axon_ntff_profile_hook = lambda: hook
    mod.set_axon_ntff_profile_hook = lambda h: None
    sys.modules["antenv.axon_hooks"] = mod


MODE = "v4"  # single variant; kept for test.py compatibility


def kernel(x, gate_w, gate_b, weight, bias):
    _ensure_ntff_hook()
    from concourse.bass_utils import run_bass_kernel_spmd

    nc = get_nc()
    in_maps = build_in_maps(x, gate_w, gate_b, weight, bias)
    res = run_bass_kernel_spmd(nc, in_maps, list(range(N_CORES)))
    return assemble_out([res.results[c]["out"] for c in range(N_CORES)])


def assemble_out(parts):
    if DEFAULT_ORIENT == "w":
        # each part is out.T as [128 i, 2*32 (blk, b)] -> (B, NOUT) per core
        cols = []
        for arr in parts:
            a = np.asarray(arr).reshape(BLK, 2, B)       # (i, blk, b)
            cols.append(a.transpose(2, 1, 0).reshape(B, NOUT))  # (b, blk*128+i)
        return np.concatenate(cols, axis=1).astype(np.float32)
    return np.concatenate(parts, axis=1).astype(np.float32)


# revision 28
# speedup vs baseline: 1.1438x; 1.0503x over previous
"""Trainium2 Bass kernel for nn_GatedBlock (moe_routing).

Math (reference collapses): the (NB,BS,BS) reshape of weight maps block k to
rows [128k, 128k+128) of weight, so
    out[b, i] = g[b, i // 128] * (x @ W.T)[b, i] + bias[i]
with g = sigmoid(x @ gate_w + gate_b), bottom-8 of 16 gates zeroed per row.

Sharding: output-dim (i) split 8 ways -> 256 rows of W (= 2 gate blocks) per
core.  Per-core inputs (k-tile-major, partition-contiguous rows):
  small (128, KT, 96) bf16  [x_hi | x_lo | gw_hi | gw_lo]
  rhs   (128, KT, 256) bf16 W_shard.T
  epi   (32, 16) f32        gate_b[perm] broadcast over batch
  epib  (128, 2) f32        bias per output block (partition = i)

Design notes (from trace analysis):
* Main matmul runs W-STATIONARY (lhsT = W k-tile [128,128], moving = x_hi
  [128,32] -> psum holds out.T).  LDWEIGHTS ingests the stationary at ~4
  cols/cycle, so a (LDW, MM) pair takes ~27ns vs ~213ns for the x-stationary
  form — W enters the PE 4x faster.  Output leaves transposed; the host
  un-transposes (32KB, trivial).
* Gate logits use an exact bf16 hi/lo split (x@gw to ~1e-5; bf16 products
  are exact, fp32 PSUM accumulate, only the x_lo*gw_lo term is dropped).
  Top-8 RANKING is done on these logits (monotonicity of sigmoid); plain
  bf16 would flip the selection (min margin 3.4e-4) which is catastrophic.
* Sigmoid VALUE comes from a degree-13 odd polynomial on the DVE (3e-4 abs
  err on the logit range).  This keeps the scalar engine activation-free:
  ACT_TABLE_LOADs were observed to stall the scalar HWDGE queue ~1.4us.
* Gating in the transposed orientation: gk rows are partition-broadcast via
  K=1 ones-matmuls (DVE 32x32 block-transposes put each block's gate row at
  partition 0 first).  Epilogue block 0 runs on DVE, block 1 on GpSimd.
* W in bf16 halves the dominant DMA (1.9e-3 rel err vs the 2e-2 gate).  The
  two HWDGE queues sustain ~130-150GB/s each concurrently; bytes are split
  so both queues finish together, with W arrival order matching the main
  matmuls' k-order consumption.
"""

import sys

for _p in ("/opt/trn_rl_repo", "/root/.axon_site/_ro/trn_rl_repo"):
    if _p not in sys.path:
        sys.path.append(_p)

import numpy as np

B = 32          # batch
D = 2048        # model dim
NB = 16         # gate blocks
BLK = D // NB   # 128 output rows per gate block
N_CORES = 8
NOUT = D // N_CORES       # 256 output cols per core
KT = D // 128             # 16 k-tiles
NSM = 96                  # small cols: 64 x (hi|lo) + 32 gw (hi|lo)

# sigmoid(t) ~= 0.5 + sum_k SIGC[k] * t^(2k+1), fitted on [-6,6] (logits for
# this problem stay within ~2.9); 2.9e-4 max err on [-4,4]
SIGC = [0.24915617679209626, -0.019605073700038885, 0.0015270501318674753,
        -8.452671874118546e-05, 2.898591921808083e-06,
        -5.3997592578994374e-08, 4.139133277746657e-10]

# DMA plan: per queue (sync=0, scalar=1), ordered entries
DEFAULT_PLAN = {
    0: [("small", 0, KT), ("epib",), ("rhs", 11, 16)],
    1: [("epi",), ("rhs", 0, 4), ("rhs", 4, 8), ("rhs", 8, 11)],
}

_compiled = {}


def _build(plan):
    import concourse.bacc as bacc
    import concourse.tile as tile
    import concourse.mybir as mybir

    f32 = mybir.dt.float32
    bf16 = mybir.dt.bfloat16
    Alu = mybir.AluOpType

    nc = bacc.Bacc("TRN2", target_bir_lowering=False, debug=False,
                   num_devices=N_CORES)

    small_d = nc.dram_tensor("small", [128, KT, NSM], bf16, kind="ExternalInput")
    rhs_d = nc.dram_tensor("rhs", [128, KT, NOUT], bf16, kind="ExternalInput")
    epi_d = nc.dram_tensor("epi", [B, NB], f32, kind="ExternalInput")
    epib_d = nc.dram_tensor("epib", [BLK, 2], f32, kind="ExternalInput")
    out_d = nc.dram_tensor("out", [BLK, 2 * B], f32, kind="ExternalOutput")

    with tile.TileContext(nc) as tc:
        with (
            tc.tile_pool(name="sb", bufs=1) as sb,
            tc.tile_pool(name="ps", bufs=1, space="PSUM") as psp,
        ):
            small = sb.tile([128, KT, NSM], bf16, name="small_sb", tag="small_sb")
            rhs = sb.tile([128, KT, NOUT], bf16, name="rhs_sb", tag="rhs_sb")
            epi = sb.tile([B, NB], f32, name="epi_sb", tag="epi_sb")
            epib = sb.tile([BLK, 2], f32, name="epib_sb", tag="epib_sb")
            t1 = sb.tile([B, NB], f32, name="t1", tag="t1")
            graw = sb.tile([B, NB], f32, name="graw", tag="graw")
            uu = sb.tile([B, NB], f32, name="uu", tag="uu")
            pp = sb.tile([B, NB], f32, name="pp", tag="pp")
            sg = sb.tile([B, NB], f32, name="sg", tag="sg")
            m8 = sb.tile([B, 8], f32, name="m8", tag="m8")
            rep = sb.tile([B, NB], f32, name="rep", tag="rep")
            dm = sb.tile([B, NB], f32, name="dm", tag="dm")
            ind = sb.tile([B, NB], f32, name="ind", tag="ind")
            gk = sb.tile([B, NB], f32, name="gk", tag="gk")
            ones = sb.tile([1, BLK], f32, name="ones", tag="ones")
            gkp = sb.tile([B, B], f32, name="gkp", tag="gkp")
            gkT = sb.tile([B, B], f32, name="gkT", tag="gkT")
            gkp1 = sb.tile([B, B], f32, name="gkp1", tag="gkp1")
            gkT1 = sb.tile([B, B], f32, name="gkT1", tag="gkT1")
            gbc = [sb.tile([BLK, B], f32, name=f"gbc{h}", tag=f"gbc{h}")
                   for h in range(2)]
            outw = sb.tile([BLK, 2 * B], f32, name="outw", tag="outw")
            ps_g = psp.tile([2 * B, 2 * NB], f32, name="ps_g", tag="ps_g")
            ps_w = [psp.tile([BLK, B], f32, name=f"ps_w{h}", tag=f"ps_w{h}")
                    for h in range(2)]
            ps_b = [psp.tile([BLK, B], f32, name=f"ps_b{h}", tag=f"ps_b{h}")
                    for h in range(2)]

            nc.gpsimd.memset(ones[:], 1.0)

            engs = [nc.sync, nc.scalar]
            for q, entries in sorted(plan.items()):
                for e in entries:
                    if e[0] == "small":
                        _, k0, k1 = e
                        engs[q].dma_start(small[:, k0:k1, :],
                                          small_d.ap()[:, k0:k1, :])
                    elif e[0] == "rhs":
                        _, k0, k1 = e
                        engs[q].dma_start(rhs[:, k0:k1, :],
                                          rhs_d.ap()[:, k0:k1, :])
                    elif e[0] == "epi":
                        engs[q].dma_start(epi[:], epi_d.ap())
                    elif e[0] == "epib":
                        engs[q].dma_start(epib[:], epib_d.ap())
                    else:
                        raise ValueError(e)

            # gate linear: M=64 ([x_hi|x_lo]) x N=32 ([gw_hi|gw_lo])
            for t in range(KT):
                nc.tensor.matmul(
                    ps_g[:], small[:, t, 0:64], small[:, t, 64:NSM],
                    start=(t == 0), stop=(t == KT - 1),
                )

            # logits = hi*hi + hi*lo + lo*hi + gate_b (each add reads at most
            # one PSUM input)
            nc.vector.tensor_add(t1[:], ps_g[0:B, 0:NB], epi[:])
            nc.vector.tensor_add(t1[:], ps_g[0:B, NB:2 * NB], t1[:])
            nc.vector.tensor_add(graw[:], ps_g[B:2 * B, 0:NB], t1[:])

            # sigmoid via odd polynomial (DVE only; no act tables):
            #   u = t^2; q = C6*u; q = (q + Ck)*u ...; s = (q + C0)*t + 0.5
            nc.vector.tensor_mul(uu[:], graw[:], graw[:])
            nc.vector.tensor_scalar_mul(pp[:], uu[:], float(SIGC[6]))
            for k in range(5, 0, -1):
                nc.vector.scalar_tensor_tensor(
                    pp[:], pp[:], float(SIGC[k]), uu[:], Alu.add, Alu.mult)
            nc.vector.scalar_tensor_tensor(
                sg[:], pp[:], float(SIGC[0]), graw[:], Alu.add, Alu.mult)
            nc.vector.tensor_scalar_add(sg[:], sg[:], 0.5)

            # top-8 mask from the exact logits: top-8 -> 1e30, then an
            # is_gt(1.0) indicator of (replaced - original)
            nc.vector.max(m8[:], graw[:])
            nc.vector.match_replace(rep[:], m8[:], graw[:], 1e30)
            nc.vector.tensor_sub(dm[:], rep[:], graw[:])
            nc.vector.tensor_scalar(ind[:], dm[:], 1.0, None, Alu.is_gt)
            nc.vector.tensor_mul(gk[:], sg[:], ind[:])

            # transpose gk so each block's gate row lands at partition 0
            # (DVE operands must start at partition 0, hence the second
            # transpose with block 1's gate column shifted into column 0)
            nc.vector.tensor_copy(gkp[:, 0:NB], gk[:])
            nc.vector.tensor_copy(gkp[:, NB:B], gk[:])
            nc.vector.transpose(gkT[:], gkp[:])
            nc.vector.tensor_copy(gkp1[:, 0:NB], gk[:])
            nc.vector.tensor_copy(gkp1[:, NB:B], gk[:])
            nc.vector.tensor_copy(gkp1[:, 0:1], gk[:, 1:2])
            nc.vector.transpose(gkT1[:], gkp1[:])

            # main matmul: W k-tile stationary (M=128), x_hi moving (N=32);
            # psum accumulates out.T per block.  The gate-broadcast K=1
            # matmuls are slotted before the last k-tile so only the final
            # W segment gates the epilogue.
            for t in range(KT):
                if t == KT - 1:
                    nc.tensor.matmul(ps_b[0][:], ones[0:1, :], gkT[0:1, 0:B],
                                     start=True, stop=True)
                    nc.tensor.matmul(ps_b[1][:], ones[0:1, :], gkT1[0:1, 0:B],
                                     start=True, stop=True)
                for h in range(2):
                    nc.tensor.matmul(
                        ps_w[h][:], rhs[:, t, h * BLK:(h + 1) * BLK],
                        small[:, t, 0:B],
                        start=(t == 0), stop=(t == KT - 1),
                    )

            # out.T = ps_w * gbc + bias (GpSimd cannot read PSUM, so both
            # blocks run on DVE; each block's store issues as soon as ready).
            # The gbc copies only need ps_b, so they run before the final
            # W-segment-gated matmuls complete.
            for h in range(2):
                nc.vector.tensor_copy(gbc[h][:], ps_b[h][:])
            for h in range(2):
                sl = slice(h * B, (h + 1) * B)
                nc.vector.tensor_mul(outw[:, sl], ps_w[h][:], gbc[h][:])
                nc.vector.tensor_scalar_add(outw[:, sl], outw[:, sl],
                                            epib[:, h:h + 1])
                engs[(h + 1) % 2].dma_start(out_d.ap()[:, sl], outw[:, sl])

    nc.compile()
    return nc


def get_nc(plan=None):
    plan = plan if plan is not None else DEFAULT_PLAN
    key = repr(sorted(plan.items()))
    if key not in _compiled:
        _compiled[key] = _build(plan)
    return _compiled[key]


def _tile_major(a):
    """(D, n) -> (128, KT, n) k-tile-major contiguous."""
    n = a.shape[1]
    return np.ascontiguousarray(a.reshape(KT, 128, n).transpose(1, 0, 2))


def _hi_lo(a):
    import ml_dtypes
    hi = a.astype(ml_dtypes.bfloat16)
    lo = (a - hi.astype(np.float32)).astype(ml_dtypes.bfloat16)
    return hi, lo


def build_in_maps(x, gate_w, gate_b, weight, bias):
    import ml_dtypes

    x = np.asarray(x, dtype=np.float32)
    gate_w = np.asarray(gate_w, dtype=np.float32)
    gate_b = np.asarray(gate_b, dtype=np.float32)
    weight = np.asarray(weight, dtype=np.float32)
    bias = np.asarray(bias, dtype=np.float32)

    x_hi, x_lo = _hi_lo(np.ascontiguousarray(x.T))               # (2048, 32)
    in_maps = []
    for c in range(N_CORES):
        perm = [2 * c, 2 * c + 1] + [k for k in range(NB)
                                     if k not in (2 * c, 2 * c + 1)]
        gw_hi, gw_lo = _hi_lo(gate_w[:, perm])                   # (2048, 16)
        small = np.concatenate([x_hi, x_lo, gw_hi, gw_lo], axis=1)  # (2048, 96)
        w_shard = np.ascontiguousarray(weight[c * NOUT:(c + 1) * NOUT, :].T)
        bs = bias[c * NOUT:(c + 1) * NOUT]
        in_maps.append({
            "small": _tile_major(small),
            "rhs": _tile_major(w_shard.astype(ml_dtypes.bfloat16)),
            "epi": np.ascontiguousarray(
                np.broadcast_to(gate_b[perm], (B, NB)).astype(np.float32)),
            "epib": np.ascontiguousarray(
                np.stack([bs[0:BLK], bs[BLK:NOUT]], axis=1).astype(np.float32)),
        })
    return in_maps


def assemble_out(parts):
    """Each part is out.T as [128 i, 2*32 (blk, b)] -> full (B, D)."""
    cols = []
    for arr in parts:
        a = np.asarray(arr).reshape(BLK, 2, B)                # (i, blk, b)
        cols.append(a.transpose(2, 1, 0).reshape(B, NOUT))    # (b, blk*128+i)
    return np.concatenate(cols, axis=1).astype(np.float32)


def _ensure_ntff_hook():
    """If a caller sets BASS_TRACE, run_bass_kernel_spmd imports
    antenv.axon_hooks, which is missing in this image; provide a working
    ctypes-backed stub so tracing degrades gracefully instead of raising."""
    try:
        from antenv.axon_hooks import get_axon_ntff_profile_hook  # noqa: F401
        return
    except ImportError:
        pass
    import contextlib
    import ctypes
    import types

    try:
        lib = ctypes.CDLL("/opt/axon/libaxon_pjrt.so")
        assert hasattr(lib, "axon_start_nrt_profile")
        lib.axon_start_nrt_profile.argtypes = [
            ctypes.POINTER(ctypes.c_int64), ctypes.c_size_t]
        lib.axon_start_nrt_profile.restype = ctypes.c_int64
        lib.axon_stop_nrt_profile.argtypes = [ctypes.c_char_p]
        lib.axon_stop_nrt_profile.restype = ctypes.c_int64

        @contextlib.contextmanager
        def _hook(output_dir, device_ids):
            import jax
            jax.devices()
            if device_ids:
                ids = (ctypes.c_int64 * len(device_ids))(*device_ids)
                rc = lib.axon_start_nrt_profile(ids, len(device_ids))
            else:
                rc = lib.axon_start_nrt_profile(None, 0)
            if rc != 0:
                raise RuntimeError(f"axon_start_nrt_profile rc={rc}")
            try:
                yield
            finally:
                lib.axon_stop_nrt_profile(str(output_dir).encode())

        hook = _hook
    except Exception:
        hook = None

    mod = types.ModuleType("antenv.axon_hooks")
    mod.get_axon_ntff_profile_hook = lambda: hook
    mod.set_axon_ntff_profile_hook = lambda h: None
    sys.modules["antenv.axon_hooks"] = mod


def kernel(x, gate_w, gate_b, weight, bias):
    _ensure_ntff_hook()
    from concourse.bass_utils import run_bass_kernel_spmd

    nc = get_nc()
    in_maps = build_in_maps(x, gate_w, gate_b, weight, bias)
    res = run_bass_kernel_spmd(nc, in_maps, list(range(N_CORES)))
    return assemble_out([res.results[c]["out"] for c in range(N_CORES)])


# revision 32
# speedup vs baseline: 1.2367x; 1.0813x over previous
"""Trainium2 Bass kernel for nn_GatedBlock (moe_routing).

Math (reference collapses): the (NB,BS,BS) reshape of weight maps block k to
rows [128k, 128k+128) of weight, so
    out[b, i] = g[b, i // 128] * (x @ W.T)[b, i] + bias[i]
with g = sigmoid(x @ gate_w + gate_b), bottom-8 of 16 gates zeroed per row.

Sharding: output-dim (i) split 8 ways -> 256 rows of W (= 2 gate blocks) per
core.  Per-core inputs (k-tile-major, partition-contiguous rows):
  small (128, KT, 96) bf16  [x_hi | x_lo | gw_hi | gw_lo]
  rhs   (128, KT, 256) bf16 W_shard.T
  epi   (32, 16) f32        gate_b[perm] broadcast over batch
  epib  (128, 2) f32        bias per output block (partition = i)

Design notes (from trace analysis):
* Main matmul runs W-STATIONARY (lhsT = W k-tile [128,128], moving = x_hi
  [128,32] -> psum holds out.T).  LDWEIGHTS ingests the stationary at ~4
  cols/cycle, so a (LDW, MM) pair takes ~27ns vs ~213ns for the x-stationary
  form — W enters the PE 4x faster.  Output leaves transposed; the host
  un-transposes (32KB, trivial).
* Gate logits use an exact bf16 hi/lo split (x@gw to ~1e-5; bf16 products
  are exact, fp32 PSUM accumulate, only the x_lo*gw_lo term is dropped).
  Top-8 RANKING is done on these logits (monotonicity of sigmoid); plain
  bf16 would flip the selection (min margin 3.4e-4) which is catastrophic.
* Sigmoid VALUE comes from a degree-13 odd polynomial on the DVE (3e-4 abs
  err on the logit range).  This keeps the scalar engine activation-free:
  ACT_TABLE_LOADs were observed to stall the scalar HWDGE queue ~1.4us.
* Gating in the transposed orientation: gk rows are partition-broadcast via
  K=1 ones-matmuls (DVE 32x32 block-transposes put each block's gate row at
  partition 0 first).  Epilogue block 0 runs on DVE, block 1 on GpSimd.
* W in bf16 halves the dominant DMA (1.9e-3 rel err vs the 2e-2 gate).  The
  two HWDGE queues sustain ~130-150GB/s each concurrently; bytes are split
  so both queues finish together, with W arrival order matching the main
  matmuls' k-order consumption.
"""

import sys

for _p in ("/opt/trn_rl_repo", "/root/.axon_site/_ro/trn_rl_repo"):
    if _p not in sys.path:
        sys.path.append(_p)

import numpy as np

B = 32          # batch
D = 2048        # model dim
NB = 16         # gate blocks
BLK = D // NB   # 128 output rows per gate block
N_CORES = 8
NOUT = D // N_CORES       # 256 output cols per core
KT = D // 128             # 16 k-tiles
NSM = 96                  # small cols: 64 x (hi|lo) + 32 gw (hi|lo)

# DMA plan: per queue (sync=0, scalar=1), ordered entries.  The scalar
# queue's start is taxed by the sigmoid ACT_TABLE fetches, so it carries
# only W; the k-ranges are ordered to match the main matmuls' consumption.
DEFAULT_PLAN = {
    0: [("small", 0, KT), ("epi",), ("epib",), ("rhs", 10, 16)],
    1: [("rhs", 0, 5), ("rhs", 5, 10)],
}

_compiled = {}


def _build(plan):
    import concourse.bacc as bacc
    import concourse.tile as tile
    import concourse.mybir as mybir

    f32 = mybir.dt.float32
    bf16 = mybir.dt.bfloat16
    Alu = mybir.AluOpType

    nc = bacc.Bacc("TRN2", target_bir_lowering=False, debug=False,
                   num_devices=N_CORES)

    small_d = nc.dram_tensor("small", [128, KT, NSM], bf16, kind="ExternalInput")
    rhs_d = nc.dram_tensor("rhs", [128, KT, NOUT], bf16, kind="ExternalInput")
    epi_d = nc.dram_tensor("epi", [B, NB], f32, kind="ExternalInput")
    epib_d = nc.dram_tensor("epib", [BLK, 2], f32, kind="ExternalInput")
    out_d = nc.dram_tensor("out", [BLK, 2 * B], f32, kind="ExternalOutput")

    with tile.TileContext(nc) as tc:
        with (
            tc.tile_pool(name="sb", bufs=1) as sb,
            tc.tile_pool(name="ps", bufs=1, space="PSUM") as psp,
        ):
            small = sb.tile([128, KT, NSM], bf16, name="small_sb", tag="small_sb")
            rhs = sb.tile([128, KT, NOUT], bf16, name="rhs_sb", tag="rhs_sb")
            epi = sb.tile([B, NB], f32, name="epi_sb", tag="epi_sb")
            epib = sb.tile([BLK, 2], f32, name="epib_sb", tag="epib_sb")
            t1 = sb.tile([B, NB], f32, name="t1", tag="t1")
            graw = sb.tile([B, NB], f32, name="graw", tag="graw")
            g = sb.tile([B, NB], f32, name="g", tag="g")
            m8 = sb.tile([B, 8], f32, name="m8", tag="m8")
            rep = sb.tile([B, NB], f32, name="rep", tag="rep")
            gk = sb.tile([B, NB], f32, name="gk", tag="gk")
            ones = sb.tile([1, BLK], bf16, name="ones", tag="ones")
            gkp = sb.tile([B, B], bf16, name="gkp", tag="gkp")
            gkT = sb.tile([B, B], bf16, name="gkT", tag="gkT")
            gkp1 = sb.tile([B, B], bf16, name="gkp1", tag="gkp1")
            gkT1 = sb.tile([B, B], bf16, name="gkT1", tag="gkT1")
            gbc = [sb.tile([BLK, B], f32, name=f"gbc{h}", tag=f"gbc{h}")
                   for h in range(2)]
            outw = sb.tile([BLK, 2 * B], f32, name="outw", tag="outw")
            ps_g = psp.tile([2 * B, 2 * NB], f32, name="ps_g", tag="ps_g")
            ps_w = [psp.tile([BLK, B], f32, name=f"ps_w{h}", tag=f"ps_w{h}")
                    for h in range(2)]
            ps_b = [psp.tile([BLK, B], f32, name=f"ps_b{h}", tag=f"ps_b{h}")
                    for h in range(2)]

            # constants staged while the DMAs stream (gkp/gkp1 are zeroed so
            # the later transposes read initialized data; only column 0 of
            # each carries a gate row)
            nc.gpsimd.memset(ones[:], 1.0)
            nc.gpsimd.memset(gkp[:], 0.0)
            nc.gpsimd.memset(gkp1[:], 0.0)

            engs = [nc.sync, nc.scalar]
            for q, entries in sorted(plan.items()):
                for e in entries:
                    if e[0] == "small":
                        _, k0, k1 = e
                        engs[q].dma_start(small[:, k0:k1, :],
                                          small_d.ap()[:, k0:k1, :])
                    elif e[0] == "rhs":
                        _, k0, k1 = e
                        engs[q].dma_start(rhs[:, k0:k1, :],
                                          rhs_d.ap()[:, k0:k1, :])
                    elif e[0] == "epi":
                        engs[q].dma_start(epi[:], epi_d.ap())
                    elif e[0] == "epib":
                        engs[q].dma_start(epib[:], epib_d.ap())
                    else:
                        raise ValueError(e)

            # gate linear: M=64 ([x_hi|x_lo]) x N=32 ([gw_hi|gw_lo])
            for t in range(KT):
                nc.tensor.matmul(
                    ps_g[:], small[:, t, 0:64], small[:, t, 64:NSM],
                    start=(t == 0), stop=(t == KT - 1),
                )

            # logits = hi*hi + hi*lo + lo*hi + gate_b (each add reads at most
            # one PSUM input)
            nc.vector.tensor_add(t1[:], ps_g[0:B, 0:NB], epi[:])
            nc.vector.tensor_add(t1[:], ps_g[0:B, NB:2 * NB], t1[:])
            nc.vector.tensor_add(graw[:], ps_g[B:2 * B, 0:NB], t1[:])
            nc.scalar.activation(g[:], graw[:],
                                 mybir.ActivationFunctionType.Sigmoid)
            nc.vector.max(m8[:], g[:])
            nc.vector.match_replace(rep[:], m8[:], g[:], 0.0)
            nc.vector.tensor_sub(gk[:], g[:], rep[:])

            # transpose gk so each block's gate row lands at partition 0
            # (DVE operands must start at partition 0, hence the second
            # transpose with block 1's gate column shifted into column 0);
            # bf16 so the downstream K=1 broadcast matmuls run single-pass
            nc.vector.tensor_copy(gkp[:, 0:1], gk[:, 0:1])
            nc.vector.transpose(gkT[:], gkp[:])
            nc.vector.tensor_copy(gkp1[:, 0:1], gk[:, 1:2])
            nc.vector.transpose(gkT1[:], gkp1[:])

            # main matmul: W k-tile stationary (M=128), x_hi moving (N=32);
            # psum accumulates out.T per block.  The gate-broadcast K=1
            # matmuls are slotted before the last k-tile so only the final
            # W segment gates the epilogue.
            for t in range(KT):
                if t == KT - 1:
                    nc.tensor.matmul(ps_b[0][:], ones[0:1, :], gkT[0:1, 0:B],
                                     start=True, stop=True)
                    nc.tensor.matmul(ps_b[1][:], ones[0:1, :], gkT1[0:1, 0:B],
                                     start=True, stop=True)
                for h in range(2):
                    nc.tensor.matmul(
                        ps_w[h][:], rhs[:, t, h * BLK:(h + 1) * BLK],
                        small[:, t, 0:B],
                        start=(t == 0), stop=(t == KT - 1),
                    )

            # out.T = ps_w * gbc + bias (GpSimd cannot read PSUM, so both
            # blocks run on DVE; each block's store issues as soon as ready).
            # The gbc copies only need ps_b, so they run before the final
            # W-segment-gated matmuls complete.
            for h in range(2):
                nc.vector.tensor_copy(gbc[h][:], ps_b[h][:])
            for h in range(2):
                sl = slice(h * B, (h + 1) * B)
                nc.vector.tensor_mul(outw[:, sl], ps_w[h][:], gbc[h][:])
                nc.vector.tensor_scalar_add(outw[:, sl], outw[:, sl],
                                            epib[:, h:h + 1])
                engs[(h + 1) % 2].dma_start(out_d.ap()[:, sl], outw[:, sl])

    nc.compile()
    return nc


def get_nc(plan=None):
    plan = plan if plan is not None else DEFAULT_PLAN
    key = repr(sorted(plan.items()))
    if key not in _compiled:
        _compiled[key] = _build(plan)
    return _compiled[key]


def _tile_major(a):
    """(D, n) -> (128, KT, n) k-tile-major contiguous."""
    n = a.shape[1]
    return np.ascontiguousarray(a.reshape(KT, 128, n).transpose(1, 0, 2))


def _hi_lo(a):
    import ml_dtypes
    hi = a.astype(ml_dtypes.bfloat16)
    lo = (a - hi.astype(np.float32)).astype(ml_dtypes.bfloat16)
    return hi, lo


def build_in_maps(x, gate_w, gate_b, weight, bias):
    import ml_dtypes

    x = np.asarray(x, dtype=np.float32)
    gate_w = np.asarray(gate_w, dtype=np.float32)
    gate_b = np.asarray(gate_b, dtype=np.float32)
    weight = np.asarray(weight, dtype=np.float32)
    bias = np.asarray(bias, dtype=np.float32)

    x_hi, x_lo = _hi_lo(np.ascontiguousarray(x.T))               # (2048, 32)
    in_maps = []
    for c in range(N_CORES):
        perm = [2 * c, 2 * c + 1] + [k for k in range(NB)
                                     if k not in (2 * c, 2 * c + 1)]
        gw_hi, gw_lo = _hi_lo(gate_w[:, perm])                   # (2048, 16)
        small = np.concatenate([x_hi, x_lo, gw_hi, gw_lo], axis=1)  # (2048, 96)
        w_shard = np.ascontiguousarray(weight[c * NOUT:(c + 1) * NOUT, :].T)
        bs = bias[c * NOUT:(c + 1) * NOUT]
        in_maps.append({
            "small": _tile_major(small),
            "rhs": _tile_major(w_shard.astype(ml_dtypes.bfloat16)),
            "epi": np.ascontiguousarray(
                np.broadcast_to(gate_b[perm], (B, NB)).astype(np.float32)),
            "epib": np.ascontiguousarray(
                np.stack([bs[0:BLK], bs[BLK:NOUT]], axis=1).astype(np.float32)),
        })
    return in_maps


def assemble_out(parts):
    """Each part is out.T as [128 i, 2*32 (blk, b)] -> full (B, D)."""
    cols = []
    for arr in parts:
        a = np.asarray(arr).reshape(BLK, 2, B)                # (i, blk, b)
        cols.append(a.transpose(2, 1, 0).reshape(B, NOUT))    # (b, blk*128+i)
    return np.concatenate(cols, axis=1).astype(np.float32)


def _ensure_ntff_hook():
    """If a caller sets BASS_TRACE, run_bass_kernel_spmd imports
    antenv.axon_hooks, which is missing in this image; provide a working
    ctypes-backed stub so tracing degrades gracefully instead of raising."""
    try:
        from antenv.axon_hooks import get_axon_ntff_profile_hook  # noqa: F401
        return
    except ImportError:
        pass
    import contextlib
    import ctypes
    import types

    try:
        lib = ctypes.CDLL("/opt/axon/libaxon_pjrt.so")
        assert hasattr(lib, "axon_start_nrt_profile")
        lib.axon_start_nrt_profile.argtypes = [
            ctypes.POINTER(ctypes.c_int64), ctypes.c_size_t]
        lib.axon_start_nrt_profile.restype = ctypes.c_int64
        lib.axon_stop_nrt_profile.argtypes = [ctypes.c_char_p]
        lib.axon_stop_nrt_profile.restype = ctypes.c_int64

        @contextlib.contextmanager
        def _hook(output_dir, device_ids):
            import jax
            jax.devices()
            if device_ids:
                ids = (ctypes.c_int64 * len(device_ids))(*device_ids)
                rc = lib.axon_start_nrt_profile(ids, len(device_ids))
            else:
                rc = lib.axon_start_nrt_profile(None, 0)
            if rc != 0:
                raise RuntimeError(f"axon_start_nrt_profile rc={rc}")
            try:
                yield
            finally:
                lib.axon_stop_nrt_profile(str(output_dir).encode())

        hook = _hook
    except Exception:
        hook = None

    mod = types.ModuleType("antenv.axon_hooks")
    mod.get_axon_ntff_profile_hook = lambda: hook
    mod.set_axon_ntff_profile_hook = lambda h: None
    sys.modules["antenv.axon_hooks"] = mod


def kernel(x, gate_w, gate_b, weight, bias):
    _ensure_ntff_hook()
    from concourse.bass_utils import run_bass_kernel_spmd

    nc = get_nc()
    in_maps = build_in_maps(x, gate_w, gate_b, weight, bias)
    res = run_bass_kernel_spmd(nc, in_maps, list(range(N_CORES)))
    return assemble_out([res.results[c]["out"] for c in range(N_CORES)])
